# revision 1
# baseline (speedup 1.0000x reference)
"""Trainium2 Bass kernel for the emoji-box decoder problem.

Math: the reference builds, per picture, a [H*W, S*S] squared-distance
matrix d2[(r,c),(i,j)] = (r - src_r[i])^2 + (c - src_c[j])^2, softmaxes
over (i,j) and applies it to the selected emoji image.  d2 is separable,
so softmax(-d2) = softmax_r (x) softmax_c exactly (the joint max shift
equals the sum of the per-axis min shifts).  The kernel therefore
computes two [.,64] row softmaxes and contracts
    R[ch] = Ar @ img_w[ch] @ Ac^T
with img_w = sum_k onehot(argmax logits)_k * images[k] computed on
device.  The box mask is an outer product rowin (x) colin, and the
output is M*(R-1) + valid (M = valid*inside), matching
where(valid, where(inside, R, 1), 0).

Sharding: 8 cores = 2 pictures x 4 row-blocks of 64 canvas rows.
images is replicated (channels 0..2 only are read); per-core xmeta
carries the picture's X row plus the row offset r0.
"""

import sys

import numpy as np

if "/opt/trn_rl_repo" not in sys.path:
    sys.path.insert(0, "/opt/trn_rl_repo")

import concourse.bacc as bacc
import concourse.bass as bass
import concourse.mybir as mybir
import concourse.tile as tile
from concourse.bass_utils import run_bass_kernel_spmd


def _ensure_ntff_hook():
    """The image's antenv package lacks axon_hooks, so trn_boot's NTFF
    profile hook install degrades silently and run_bass_kernel_spmd
    crashes on `from antenv.axon_hooks import ...` when trace=True.
    Provide the module and install the ctypes hook ourselves."""
    import types

    try:
        from antenv.axon_hooks import get_axon_ntff_profile_hook  # noqa: F401

        return
    except ImportError:
        pass
    mod = types.ModuleType("antenv.axon_hooks")
    _hook = [None]
    mod.set_axon_ntff_profile_hook = lambda h: _hook.__setitem__(0, h)
    mod.get_axon_ntff_profile_hook = lambda: _hook[0]
    try:
        import antenv

        sys.modules["antenv.axon_hooks"] = mod
        antenv.axon_hooks = mod
        from trn_agent_boot.trn_boot import _ntff_profile_via_ctypes

        hook = _ntff_profile_via_ctypes("/opt/axon/libaxon_pjrt.so")
        if hook is not None:
            mod.set_axon_ntff_profile_hook(hook)
    except Exception:
        pass


_ensure_ntff_hook()

F32 = mybir.dt.float32
I32 = mybir.dt.int32
AF = mybir.ActivationFunctionType
OP = mybir.AluOpType
AX = mybir.AxisListType

MAGIC = 8388608.0  # 2**23; x + MAGIC - MAGIC == rint(x) for 0 <= x < 2**22

N_CORES = 8
H = 256
S = 64
N_IMG = 14
RB = 64  # canvas rows per core


def build_nc():
    nc = bacc.Bacc("TRN2", target_bir_lowering=False, debug=False)

    xmeta_d = nc.dram_tensor("xmeta", [1, 20], F32, kind="ExternalInput")
    images_d = nc.dram_tensor("images", [N_IMG, 4, S, S], F32, kind="ExternalInput")
    out_d = nc.dram_tensor("out", [3, RB, H], F32, kind="ExternalOutput")
    idx_d = nc.dram_tensor("idx_bounce", [1, 1], I32)

    with tile.TileContext(nc) as tc:
        with (
            tc.tile_pool(name="constp", bufs=1) as constp,
            tc.tile_pool(name="workp", bufs=2) as workp,
            tc.tile_pool(name="outp", bufs=1) as outp,
            tc.tile_pool(name="ps_tp", bufs=2, space="PSUM") as ps_tp,
            tc.tile_pool(name="ps_t1", bufs=2, space="PSUM") as ps_t1,
            tc.tile_pool(name="ps_r", bufs=1, space="PSUM") as ps_r,
        ):
            # ---- warm the scalar-engine activation table early so the
            # ~1.3us ACT_TABLE_LOAD overlaps the input DMA
            warm = workp.tile([1, 1], F32)
            nc.gpsimd.memset(warm[:], 0.0)
            warm2 = workp.tile([1, 1], F32)
            nc.scalar.activation(warm2[:], warm[:], AF.Exp)

            # ---- xmeta -> broadcast to all 128 partitions via PE outer product
            xrow = constp.tile([1, 20], F32)
            nc.sync.dma_start(xrow[:], xmeta_d[:])
            ones_row = constp.tile([1, 128], F32)
            nc.vector.memset(ones_row[:], 1.0)
            xb_ps = ps_tp.tile([128, 20], F32, tag="tp")
            nc.tensor.matmul(xb_ps[:], ones_row[:], xrow[:])
            xb = constp.tile([128, 20], F32)
            nc.scalar.copy(xb[:], xb_ps[:])

            # ---- emoji index chain FIRST: argmax -> register -> gather DMA
            lmax = workp.tile([1, 1], F32)
            nc.vector.tensor_reduce(lmax[:], xb[0:1, 5:19], AX.X, OP.max)
            wb = workp.tile([1, N_IMG], F32)
            nc.vector.tensor_scalar(wb[:], xb[0:1, 5:19], lmax[:], None, OP.is_ge)
            iota14i = constp.tile([1, N_IMG], I32)
            nc.gpsimd.iota(iota14i[:], pattern=[[1, N_IMG]], base=0, channel_multiplier=0)
            iota14f = constp.tile([1, N_IMG], F32)
            nc.vector.tensor_copy(iota14f[:], iota14i[:])
            dotj = workp.tile([1, N_IMG], F32)
            nc.vector.tensor_tensor(dotj[:], wb[0:1, :], iota14f[:], OP.mult)
            idxf = workp.tile([1, 1], F32)
            nc.vector.tensor_reduce(idxf[:], dotj[:], AX.X, OP.add)
            idxi = workp.tile([1, 1], I32)
            nc.vector.tensor_copy(idxi[:], idxf[:])
            wimg = constp.tile([64, 3, 64], F32)
            with nc.gpsimd.register("ridx") as ridx:
                nc.gpsimd.reg_load(ridx, idxi[0:1, 0:1])
                off = nc.gpsimd.snap(ridx)
                nc.gpsimd.dma_start(
                    wimg[:],
                    images_d[bass.ds(off, 1), 0:3, :, :].squeeze(0).transpose([1, 0, 2]),
                )

            # ---- rounded box coords cs = rint(256 * X[0:4]) (mult by 256 is
            # exact, so fusing mult+magic-add keeps rint semantics)
            cs = constp.tile([128, 4], F32)
            nc.vector.tensor_scalar(cs[:], xb[:, 0:4], 256.0, MAGIC, OP.mult, OP.add)
            nc.vector.tensor_scalar(cs[:], cs[:], MAGIC, None, OP.subtract)

            # ---- valid flag = AND of six conditions (min of 0/1 flags)
            vfl = workp.tile([128, 6], F32)
            nc.vector.tensor_scalar(vfl[:, 0:1], cs[:, 0:1], 0.0, None, OP.is_ge)
            nc.vector.tensor_scalar(vfl[:, 1:2], cs[:, 2:3], 0.0, None, OP.is_ge)
            nc.vector.tensor_scalar(vfl[:, 2:3], cs[:, 1:2], 256.0, None, OP.is_le)
            nc.vector.tensor_scalar(vfl[:, 3:4], cs[:, 3:4], 256.0, None, OP.is_le)
            nc.vector.tensor_tensor(vfl[:, 4:5], cs[:, 1:2], cs[:, 0:1], OP.is_gt)
            nc.vector.tensor_tensor(vfl[:, 5:6], cs[:, 3:4], cs[:, 2:3], OP.is_gt)
            valid = constp.tile([128, 1], F32)
            nc.vector.tensor_reduce(valid[:], vfl[:], AX.X, OP.min)

            # ---- box_r/64, box_c/64 (exact: integer diff times 2^-6)
            c64 = constp.tile([128, 1], F32)
            nc.vector.memset(c64[:], 1.0 / 64.0)
            boxr64 = constp.tile([128, 1], F32)
            nc.vector.scalar_tensor_tensor(
                boxr64[:], cs[:, 1:2], cs[:, 0:1], c64[:], OP.subtract, OP.mult
            )
            boxc64 = constp.tile([128, 1], F32)
            nc.vector.scalar_tensor_tensor(
                boxc64[:], cs[:, 3:4], cs[:, 2:3], c64[:], OP.subtract, OP.mult
            )


            # ---- iotas / identity
            iota_pi = constp.tile([128, 1], I32)
            nc.gpsimd.iota(iota_pi[:], pattern=[[1, 1]], base=0, channel_multiplier=1)
            iota_pf = constp.tile([128, 1], F32)
            nc.vector.tensor_copy(iota_pf[:], iota_pi[:])
            iota64i = constp.tile([128, 64], I32)
            nc.gpsimd.iota(iota64i[:], pattern=[[1, 64]], base=0, channel_multiplier=0)
            iota64f = constp.tile([128, 64], F32)
            nc.vector.tensor_copy(iota64f[:], iota64i[:])
            iota256i = constp.tile([1, 256], I32)
            nc.gpsimd.iota(iota256i[:], pattern=[[1, 256]], base=0, channel_multiplier=0)
            iota256f = constp.tile([1, 256], F32)
            nc.vector.tensor_copy(iota256f[:], iota256i[:])

            ones128 = workp.tile([128, 128], F32)
            nc.gpsimd.memset(ones128[:], 1.0)
            id128 = constp.tile([128, 128], F32)
            nc.gpsimd.affine_select(
                id128[:],
                ones128[:],
                pattern=[[1, 128]],
                compare_op=OP.is_equal,
                fill=0.0,
                base=0,
                channel_multiplier=-1,
            )

            # ---- Ar: row softmax for this core's 64 canvas rows
            r_idx = workp.tile([64, 1], F32)
            nc.vector.tensor_scalar(r_idx[:], iota_pf[0:64, :], xb[0:64, 19:20], None, OP.add)
            srcr = workp.tile([64, 64], F32)
            nc.vector.tensor_scalar(
                srcr[:], iota64f[0:64, :], boxr64[0:64, :], cs[0:64, 0:1], OP.mult, OP.add
            )
            dr = workp.tile([64, 64], F32)
            nc.vector.tensor_scalar(dr[:], srcr[:], r_idx[:], None, OP.subtract)
            dr2 = workp.tile([64, 64], F32)
            nc.scalar.square(dr2[:], dr[:])
            rmin = workp.tile([64, 1], F32)
            nc.vector.tensor_reduce(rmin[:], dr2[:], AX.X, OP.min)
            er = workp.tile([64, 64], F32)
            zr = workp.tile([64, 1], F32)
            nc.scalar.activation(er[:], dr2[:], AF.Exp, bias=rmin[:], scale=-1.0, accum_out=zr[:])
            rzr = workp.tile([64, 1], F32)
            nc.vector.reciprocal(rzr[:], zr[:])
            Ar = workp.tile([64, 64], F32)
            nc.vector.tensor_scalar(Ar[:], er[:], rzr[:], None, OP.mult)

            # ArT via PE (normal f32 matmul against identity: Ar.T @ I)
            arT_ps = ps_tp.tile([64, 64], F32, tag="tp")
            nc.tensor.matmul(arT_ps[:], Ar[:], id128[0:64, 0:64])
            ArT = constp.tile([64, 64], F32)
            nc.scalar.copy(ArT[:], arT_ps[:])

            # ---- Ac: column softmax, both 128-column halves, then transpose
            srcc = workp.tile([128, 64], F32)
            nc.vector.tensor_scalar(
                srcc[:], iota64f[:], boxc64[:], cs[:, 2:3], OP.mult, OP.add
            )
            AcT = constp.tile([64, 256], F32)
            for t in range(2):
                c_idx = workp.tile([128, 1], F32, tag="c_idx")
                nc.vector.tensor_scalar(c_idx[:], iota_pf[:], float(128 * t), None, OP.add)
                dc = workp.tile([128, 64], F32, tag="dc")
                nc.vector.tensor_scalar(dc[:], srcc[:], c_idx[:], None, OP.subtract)
                dc2 = workp.tile([128, 64], F32, tag="dc2")
                nc.scalar.square(dc2[:], dc[:])
                cmin = workp.tile([128, 1], F32, tag="cmin")
                nc.vector.tensor_reduce(cmin[:], dc2[:], AX.X, OP.min)
                ec = workp.tile([128, 64], F32, tag="ec")
                zc = workp.tile([128, 1], F32, tag="zc")
                nc.scalar.activation(
                    ec[:], dc2[:], AF.Exp, bias=cmin[:], scale=-1.0, accum_out=zc[:]
                )
                rzc = workp.tile([128, 1], F32, tag="rzc")
                nc.vector.reciprocal(rzc[:], zc[:])
                Ac = workp.tile([128, 64], F32, tag="Ac")
                nc.vector.tensor_scalar(Ac[:], ec[:], rzc[:], None, OP.mult)
                acT_ps = ps_tp.tile([64, 128], F32, tag="tp")
                nc.tensor.matmul(acT_ps[:], Ac[:], id128[:])
                nc.scalar.copy(AcT[:, 128 * t : 128 * (t + 1)], acT_ps[:])


            # ---- box mask M = (valid * rowin) (x) colin via PE outer product

            # ---- contraction: T1[ch] then R with ch0|ch1 stacked in one matmul
            T1all = constp.tile([64, 192], F32)
            for ch in range(3):
                t1_ps = ps_t1.tile([64, 64], F32, tag="t1", name=f"t1ps{ch}")
                nc.tensor.matmul(t1_ps[:], wimg[:, ch, :], ArT[:])
                nc.vector.tensor_copy(T1all[:, 64 * ch : 64 * (ch + 1)], t1_ps[:])
            r_ab_ps = ps_r.tile([128, 256], F32, tag="rab", bufs=1)
            nc.tensor.matmul(r_ab_ps[:], T1all[:, 0:128], AcT[:])
            r_c_ps = ps_r.tile([64, 256], F32, tag="rc", bufs=1)
            nc.tensor.matmul(r_c_ps[:], T1all[:, 128:192], AcT[:])

            ridx_row = workp.tile([1, 2, 64], F32)
            nc.vector.tensor_scalar(ridx_row[0:1, 0, :], iota64f[0:1, :], xb[0:1, 19:20], None, OP.add)
            nc.vector.tensor_scalar(ridx_row[0:1, 1, :], iota64f[0:1, :], xb[0:1, 19:20], None, OP.add)
            rowin = workp.tile([1, 128], F32)
            rtmp = workp.tile([1, 128], F32)
            nc.vector.tensor_scalar(rowin[:], ridx_row[0:1, :, :], cs[0:1, 0:1], None, OP.is_ge)
            nc.vector.tensor_scalar(rtmp[:], ridx_row[0:1, :, :], cs[0:1, 1:2], None, OP.is_lt)
            nc.vector.tensor_tensor(rowin[:], rowin[:], rtmp[:], OP.mult)
            nc.vector.tensor_scalar(rowin[:], rowin[:], valid[0:1, :], None, OP.mult)
            colin = workp.tile([1, 256], F32)
            ctmp = workp.tile([1, 256], F32)
            nc.vector.tensor_scalar(colin[:], iota256f[:], cs[0:1, 2:3], None, OP.is_ge)
            nc.vector.tensor_scalar(ctmp[:], iota256f[:], cs[0:1, 3:4], None, OP.is_lt)
            nc.vector.tensor_tensor(colin[:], colin[:], ctmp[:], OP.mult)
            m_ps = ps_tp.tile([128, 256], F32, tag="m", bufs=1)
            nc.tensor.matmul(m_ps[:], rowin[:], colin[:])
            m_sb = constp.tile([128, 256], F32)
            nc.vector.tensor_copy(m_sb[:], m_ps[:])

            res_ab = outp.tile([128, 256], F32)
            nc.vector.scalar_tensor_tensor(
                res_ab[:], r_ab_ps[:], -1.0, m_sb[:], OP.add, OP.mult
            )
            nc.vector.tensor_scalar(res_ab[:], res_ab[:], valid[:], None, OP.add)
            res_c = outp.tile([64, 256], F32)
            nc.vector.scalar_tensor_tensor(
                res_c[:], r_c_ps[:], -1.0, m_sb[0:64, :], OP.add, OP.mult
            )
            nc.vector.tensor_scalar(res_c[:], res_c[:], valid[0:64, :], None, OP.add)
            nc.sync.dma_start(
                out_d[0:2, :, :].rearrange("a b c -> (a b) c"), res_ab[:]
            )
            nc.gpsimd.dma_start(out_d[2, :, :], res_c[:])

    nc.compile()
    return nc


_CACHE = {}


def get_nc():
    if "nc" not in _CACHE:
        _CACHE["nc"] = build_nc()
    return _CACHE["nc"]


def make_in_maps(X, images):
    X = np.ascontiguousarray(np.asarray(X, np.float32))
    images = np.ascontiguousarray(np.asarray(images, np.float32))
    in_maps = []
    for c in range(N_CORES):
        pic, rb = divmod(c, 4)
        xm = np.zeros((1, 20), np.float32)
        xm[0, :19] = X[pic, 0]
        xm[0, 19] = float(RB * rb)
        in_maps.append({"xmeta": xm, "images": images})
    return in_maps


def assemble(results):
    out = np.empty((2, 3, H, H), np.float32)
    for c in range(N_CORES):
        pic, rb = divmod(c, 4)
        out[pic, :, RB * rb : RB * (rb + 1), :] = results[c]["out"]
    return out


def _axon_reset():
    try:
        import ctypes

        import jax

        jax.devices()
        ctypes.CDLL("/opt/axon/libaxon_pjrt.so").axon_reset()
    except Exception:
        pass


def kernel(X, images):
    nc = get_nc()
    in_maps = make_in_maps(X, images)
    try:
        res = run_bass_kernel_spmd(nc, in_maps, list(range(N_CORES)))
    except Exception:
        # the axon terminal can be left in a bad state by earlier failed
        # runs (LoadExecutable errors); reset and retry once
        _axon_reset()
        res = run_bass_kernel_spmd(nc, in_maps, list(range(N_CORES)))
    return assemble(res.results)

